# revision 5
# baseline (speedup 1.0000x reference)
"""BERT self-attention (B=4, S=2048, H=768, 12 heads x d=64) on 8 Trainium2
NeuronCores.

Sharding: core c handles batch b = c//2 and head group hg = c%2 (6 heads).
Each core computes q/k/v projections for its 6 heads from its batch's
hidden_states, then attention per head. No cross-core communication; the host
scatters inputs and gathers/reassembles the output.

Per-core layouts (SBUF is [128 partitions x free]):
  xT   [128, 7, 2048]  x[b].T padded to 896 rows; row 768 = ones (bias fold)
  wq/wk/wv [128, 7, 384]  weight column-slices, row 768 = bias
  qT/kT [128, 3, 2048]  per head-pair stacked d-dims (head even: p0-63,
                        odd: p64-127); computed as W.T @ x.T via PE
  v    [128, 16, 6, 65] token-major v per j-tile; col 64 of each head = ones
                        (yields sumexp for free during the ctx matmul)
  scoresT psum [128 j, 2, 512 i] -> exp -> expT sbuf [128, 16, 512] bf16
  ctx^T psum [65, 512]: rows 0-63 = unnormalized ctx^T, row 64 = sumexp;
  reciprocal -> PE outer-product broadcast into rows 64-127 -> DVE multiply.

Matmul dtypes: projections and scores in float32r (full-rate, ~tf32
precision); probs (expT) and v in bf16 for the ctx matmul.
"""
import numpy as np

import concourse.bass as bass
import concourse.mybir as mybir
import concourse.tile as tile
from concourse import bacc
from concourse.bass import ts
from concourse.bass_utils import run_bass_kernel_spmd

F32R = mybir.dt.float32r
F32 = mybir.dt.float32
BF16 = mybir.dt.bfloat16

HIDDEN = 768
N_HEADS = 12
HEAD_DIM = 64
B = 4
S = 2048
HPC = 6          # heads per core
KC = 7           # contraction chunks of 128 (768 data + bias row + pad)
NI = S // 512    # 4 i-chunks of 512
NJ = S // 128    # 16 j-tiles of 128
VW = 96  # v (64) | ones (1) | zeros (31): 32-aligned matmul M

_cache = {}
last_results = None


def _build(use_mask: bool):
    nc = bacc.Bacc("TRN2", target_bir_lowering=False, debug=False, num_devices=8)

    xT_d = nc.dram_tensor("xT", [KC * 128, S], F32R, kind="ExternalInput")
    wq_d = nc.dram_tensor("wq", [KC * 128, HPC * HEAD_DIM], F32R, kind="ExternalInput")
    wk_d = nc.dram_tensor("wk", [KC * 128, HPC * HEAD_DIM], F32R, kind="ExternalInput")
    wv_d = nc.dram_tensor("wv", [KC * 128, HPC * HEAD_DIM], F32R, kind="ExternalInput")
    cst_d = nc.dram_tensor("cst", [1, HEAD_DIM], F32R, kind="ExternalInput")
    if use_mask:
        em_d = nc.dram_tensor("em", [128, NJ], F32, kind="ExternalInput")
    out_d = nc.dram_tensor("out", [HPC, HEAD_DIM, S], F32, kind="ExternalOutput")

    with tile.TileContext(nc) as tc:
        with (
            tc.tile_pool(name="const", bufs=1) as cpool,
            tc.tile_pool(name="qk", bufs=1) as qkpool,
            tc.tile_pool(name="vp", bufs=1) as vpool,
            tc.tile_pool(name="ex", bufs=2) as expool,
            tc.tile_pool(name="op", bufs=3) as opool,
            tc.tile_pool(name="rp", bufs=2) as rpool,
            tc.tile_pool(name="pss", bufs=3, space="PSUM") as pss,
            tc.tile_pool(name="psc", bufs=2, space="PSUM") as psc,
        ):
            xT = cpool.tile([128, KC, S], F32R)
            wq = cpool.tile([128, KC, HPC * HEAD_DIM], F32R)
            wk = cpool.tile([128, KC, HPC * HEAD_DIM], F32R)
            wv = cpool.tile([128, KC, HPC * HEAD_DIM], F32R)
            ones = cpool.tile([1, HEAD_DIM], F32R)
            nc.sync.dma_start(ones[:], cst_d[:])
            if use_mask:
                em = cpool.tile([128, NJ], F32)
                nc.sync.dma_start(em[:], em_d[:])

            for c in range(KC):
                nc.sync.dma_start(xT[:, c, :], xT_d[ts(c, 128), :])
            for c in range(KC):
                nc.sync.dma_start(wq[:, c, :], wq_d[ts(c, 128), :])
                nc.sync.dma_start(wk[:, c, :], wk_d[ts(c, 128), :])
                nc.sync.dma_start(wv[:, c, :], wv_d[ts(c, 128), :])

            qT = qkpool.tile([128, HPC // 2, S], F32R)
            kT = qkpool.tile([128, HPC // 2, S], F32R)
            v = vpool.tile([128, NJ, HPC, VW], BF16)
            nc.vector.memset(v[:, :, :, HEAD_DIM:HEAD_DIM + 1], 1.0)
            nc.vector.memset(v[:, :, :, HEAD_DIM + 1:VW], 0.0)

            # q/k projections: psum [128 (pair d-dims), 512 tokens]
            for p in range(HPC // 2):
                for n in range(S // 512):
                    pq = psc.tile([128, 512], F32, tag="c")
                    for c in range(KC):
                        nc.tensor.matmul(
                            pq[:], wq[:, c, ts(p, 128)], xT[:, c, ts(n, 512)],
                            start=(c == 0), stop=(c == KC - 1),
                        )
                    nc.vector.tensor_copy(qT[:, p, ts(n, 512)], pq[:])
                    pk = psc.tile([128, 512], F32, tag="c")
                    for c in range(KC):
                        nc.tensor.matmul(
                            pk[:], wk[:, c, ts(p, 128)], xT[:, c, ts(n, 512)],
                            start=(c == 0), stop=(c == KC - 1),
                        )
                    nc.vector.tensor_copy(kT[:, p, ts(n, 512)], pk[:])

            # v projection: psum [128 tokens, 384]
            for jt in range(NJ):
                pv = psc.tile([128, HPC * HEAD_DIM], F32, tag="c")
                for c in range(KC):
                    nc.tensor.matmul(
                        pv[:], xT[:, c, ts(jt, 128)], wv[:, c, :],
                        start=(c == 0), stop=(c == KC - 1),
                    )
                nc.vector.tensor_copy(
                    v[:, jt, :, 0:HEAD_DIM],
                    pv[:].rearrange("p (h e) -> p h e", h=HPC),
                )

            # attention
            for h in range(HPC):
                p = h // 2
                po = 64 * (h % 2)
                for ic in range(NI):
                    ex = expool.tile([128, NJ, 512], BF16, tag="e")
                    for r in range(NJ // 2):
                        ss = pss.tile([128, 2, 512], F32, tag="s")
                        for jj in range(2):
                            jt = 2 * r + jj
                            nc.tensor.matmul(
                                ss[:, jj, :],
                                kT[po:po + 64, p, ts(jt, 128)],
                                qT[po:po + 64, p, ts(ic, 512)],
                                start=True, stop=True,
                            )
                        nc.scalar.activation(
                            ex[:, 2 * r:2 * r + 2, :], ss[:],
                            mybir.ActivationFunctionType.Exp,
                            scale=1.0 / np.sqrt(HEAD_DIM),
                        )
                        if use_mask:
                            for jj in range(2):
                                jt = 2 * r + jj
                                nc.vector.tensor_scalar_mul(
                                    ex[:, jt, :], ex[:, jt, :], em[:, jt:jt + 1]
                                )
                    pc = psc.tile([128, 512], F32, tag="c")
                    for jt in range(NJ):
                        nc.tensor.matmul(
                            pc[0:VW, :], v[:, jt, h, :], ex[:, jt, :],
                            start=(jt == 0), stop=(jt == NJ - 1),
                        )
                    rc = rpool.tile([1, 512], F32R)
                    with nc.allow_low_precision(reason="fp32r reciprocal"):
                        nc.vector.reciprocal(rc[:], pc[64:65, :])
                    pr = pss.tile([64, 512], F32, tag="s")
                    nc.tensor.matmul(pr[:], ones[:], rc[:], start=True, stop=True)
                    rb = opool.tile([64, 512], F32, tag="rb")
                    nc.vector.tensor_copy(rb[:], pr[:])
                    o = opool.tile([64, 512], F32, tag="o")
                    nc.vector.tensor_tensor(
                        o[:], pc[0:64, :], rb[:], op=mybir.AluOpType.mult
                    )
                    nc.sync.dma_start(out_d[h, :, ts(ic, 512)], o[:])

    nc.compile()
    return nc


def _get_nc(use_mask: bool):
    if use_mask not in _cache:
        _cache[use_mask] = _build(use_mask)
    return _cache[use_mask]


def kernel(hidden_states, attention_mask, Wq, bq, Wk, bk, Wv, bv):
    global last_results
    hidden_states = np.asarray(hidden_states, dtype=np.float32)
    attention_mask = np.asarray(attention_mask, dtype=np.float32)
    Wq = np.asarray(Wq, dtype=np.float32)
    Wk = np.asarray(Wk, dtype=np.float32)
    Wv = np.asarray(Wv, dtype=np.float32)
    bq = np.asarray(bq, dtype=np.float32)
    bk = np.asarray(bk, dtype=np.float32)
    bv = np.asarray(bv, dtype=np.float32)

    use_mask = bool(np.any(attention_mask))
    nc = _get_nc(use_mask)

    in_maps = []
    for c in range(8):
        b = c // 2
        hg = c % 2
        cs = slice(hg * HPC * HEAD_DIM, (hg + 1) * HPC * HEAD_DIM)

        xT = np.zeros((KC * 128, S), dtype=np.float32)
        xT[:HIDDEN] = hidden_states[b].T
        xT[HIDDEN] = 1.0

        def wslice(W, bias):
            w = np.zeros((KC * 128, HPC * HEAD_DIM), dtype=np.float32)
            w[:HIDDEN] = W[:, cs]
            w[HIDDEN] = bias[cs]
            return w

        m = {
            "xT": xT,
            "cst": np.ones((1, HEAD_DIM), dtype=np.float32),
            "wq": wslice(Wq, bq),
            "wk": wslice(Wk, bk),
            "wv": wslice(Wv, bv),
        }
        if use_mask:
            em = np.exp(attention_mask[b, 0, 0, :]).astype(np.float32)
            m["em"] = np.ascontiguousarray(em.reshape(NJ, 128).T)
        in_maps.append(m)

    res = run_bass_kernel_spmd(nc, in_maps, list(range(8)))
    last_results = res

    out = np.empty((B, S, HIDDEN), dtype=np.float32)
    for c in range(8):
        b = c // 2
        hg = c % 2
        r = res.results[c]["out"]  # [6, 64, 2048]
        out[b, :, hg * HPC * HEAD_DIM:(hg + 1) * HPC * HEAD_DIM] = (
            r.transpose(2, 0, 1).reshape(S, HPC * HEAD_DIM)
        )
    return out


# revision 6
# speedup vs baseline: 1.0582x; 1.0582x over previous
"""BERT self-attention (B=4, S=2048, H=768, 12 heads x d=64) on 8 Trainium2
NeuronCores.

Sharding: core c handles batch b = c//2 and head group hg = c%2 (6 heads).
Each core computes q/k/v projections for its 6 heads from its batch's
hidden_states, then attention per head. No cross-core communication; the host
scatters inputs and gathers/reassembles the output.

Per-core layouts (SBUF is [128 partitions x free]):
  xT   [128, 7, 2048]  x[b].T padded to 896 rows; row 768 = ones (bias fold)
  wq/wk/wv [128, 7, 384]  weight column-slices, row 768 = bias
  qT/kT [128, 3, 2048]  per head-pair stacked d-dims (head even: p0-63,
                        odd: p64-127); computed as W.T @ x.T via PE
  v    [128, 16, 6, 65] token-major v per j-tile; col 64 of each head = ones
                        (yields sumexp for free during the ctx matmul)
  scoresT psum [128 j, 2, 512 i] -> exp -> expT sbuf [128, 16, 512] bf16
  ctx^T psum [65, 512]: rows 0-63 = unnormalized ctx^T, row 64 = sumexp;
  reciprocal -> PE outer-product broadcast into rows 64-127 -> DVE multiply.

Matmul dtypes: projections and scores in float32r (full-rate, ~tf32
precision); probs (expT) and v in bf16 for the ctx matmul.
"""
import numpy as np

import concourse.bass as bass
import concourse.mybir as mybir
import concourse.tile as tile
from concourse import bacc
from concourse.bass import ts
from concourse.bass_utils import run_bass_kernel_spmd

F32R = mybir.dt.float32r
F32 = mybir.dt.float32
BF16 = mybir.dt.bfloat16

HIDDEN = 768
N_HEADS = 12
HEAD_DIM = 64
B = 4
S = 2048
HPC = 6          # heads per core
KC = 7           # contraction chunks of 128 (768 data + bias row + pad)
NI = S // 512    # 4 i-chunks of 512
NJ = S // 128    # 16 j-tiles of 128
VW = 96  # v (64) | ones (1) | zeros (31): 32-aligned matmul M

_cache = {}
last_results = None


def _build(use_mask: bool):
    nc = bacc.Bacc("TRN2", target_bir_lowering=False, debug=False, num_devices=8)

    xT_d = nc.dram_tensor("xT", [KC * 128, S], F32R, kind="ExternalInput")
    wq_d = nc.dram_tensor("wq", [KC * 128, HPC * HEAD_DIM], F32R, kind="ExternalInput")
    wk_d = nc.dram_tensor("wk", [KC * 128, HPC * HEAD_DIM], F32R, kind="ExternalInput")
    wv_d = nc.dram_tensor("wv", [KC * 128, HPC * HEAD_DIM], F32R, kind="ExternalInput")
    cst_d = nc.dram_tensor("cst", [1, HEAD_DIM], F32R, kind="ExternalInput")
    if use_mask:
        em_d = nc.dram_tensor("em", [128, NJ], F32, kind="ExternalInput")
    out_d = nc.dram_tensor("out", [HPC, HEAD_DIM, S], F32, kind="ExternalOutput")

    with tile.TileContext(nc) as tc:
        with (
            tc.tile_pool(name="const", bufs=1) as cpool,
            tc.tile_pool(name="qk", bufs=1) as qkpool,
            tc.tile_pool(name="vp", bufs=1) as vpool,
            tc.tile_pool(name="ex", bufs=2) as expool,
            tc.tile_pool(name="op", bufs=3) as opool,
            tc.tile_pool(name="rp", bufs=2) as rpool,
            tc.tile_pool(name="pss", bufs=3, space="PSUM") as pss,
            tc.tile_pool(name="psc", bufs=2, space="PSUM") as psc,
        ):
            xT = cpool.tile([128, KC, S], F32R)
            wq = cpool.tile([128, KC, HPC * HEAD_DIM], F32R)
            wk = cpool.tile([128, KC, HPC * HEAD_DIM], F32R)
            wv = cpool.tile([128, KC, HPC * HEAD_DIM], F32R)
            ones = cpool.tile([1, HEAD_DIM], F32R)
            nc.sync.dma_start(ones[:], cst_d[:])
            if use_mask:
                em = cpool.tile([128, NJ], F32)
                nc.sync.dma_start(em[:], em_d[:])

            for c in range(KC):
                nc.sync.dma_start(xT[:, c, :], xT_d[ts(c, 128), :])
            for c in range(KC):
                nc.sync.dma_start(wq[:, c, :], wq_d[ts(c, 128), :])
                nc.sync.dma_start(wk[:, c, :], wk_d[ts(c, 128), :])
                nc.sync.dma_start(wv[:, c, :], wv_d[ts(c, 128), :])

            qT = qkpool.tile([128, HPC // 2, S], BF16)
            kT = qkpool.tile([128, HPC // 2, S], BF16)
            v = vpool.tile([128, NJ, HPC, VW], BF16)
            nc.vector.memset(v[:, :, :, HEAD_DIM:HEAD_DIM + 1], 1.0)
            nc.vector.memset(v[:, :, :, HEAD_DIM + 1:VW], 0.0)

            # q/k projections: psum [128 (pair d-dims), 512 tokens]
            for p in range(HPC // 2):
                for n in range(S // 512):
                    pq = psc.tile([128, 512], F32, tag="c")
                    for c in range(KC):
                        nc.tensor.matmul(
                            pq[:], wq[:, c, ts(p, 128)], xT[:, c, ts(n, 512)],
                            start=(c == 0), stop=(c == KC - 1),
                        )
                    nc.vector.tensor_copy(qT[:, p, ts(n, 512)], pq[:])
                    pk = psc.tile([128, 512], F32, tag="c")
                    for c in range(KC):
                        nc.tensor.matmul(
                            pk[:], wk[:, c, ts(p, 128)], xT[:, c, ts(n, 512)],
                            start=(c == 0), stop=(c == KC - 1),
                        )
                    nc.vector.tensor_copy(kT[:, p, ts(n, 512)], pk[:])

            # v projection: psum [128 tokens, 384]
            for jt in range(NJ):
                pv = psc.tile([128, HPC * HEAD_DIM], F32, tag="c")
                for c in range(KC):
                    nc.tensor.matmul(
                        pv[:], xT[:, c, ts(jt, 128)], wv[:, c, :],
                        start=(c == 0), stop=(c == KC - 1),
                    )
                nc.vector.tensor_copy(
                    v[:, jt, :, 0:HEAD_DIM],
                    pv[:].rearrange("p (h e) -> p h e", h=HPC),
                )

            # attention
            for h in range(HPC):
                p = h // 2
                po = 64 * (h % 2)
                for ic in range(NI):
                    ex = expool.tile([128, NJ, 512], BF16, tag="e")
                    for r in range(NJ // 2):
                        ss = pss.tile([128, 2, 512], F32, tag="s")
                        for jj in range(2):
                            jt = 2 * r + jj
                            nc.tensor.matmul(
                                ss[:, jj, :],
                                kT[po:po + 64, p, ts(jt, 128)],
                                qT[po:po + 64, p, ts(ic, 512)],
                                start=True, stop=True,
                            )
                        nc.scalar.activation(
                            ex[:, 2 * r:2 * r + 2, :], ss[:],
                            mybir.ActivationFunctionType.Exp,
                            scale=1.0 / np.sqrt(HEAD_DIM),
                        )
                        if use_mask:
                            for jj in range(2):
                                jt = 2 * r + jj
                                nc.vector.tensor_scalar_mul(
                                    ex[:, jt, :], ex[:, jt, :], em[:, jt:jt + 1]
                                )
                    pc = psc.tile([128, 512], F32, tag="c")
                    for jt in range(NJ):
                        nc.tensor.matmul(
                            pc[0:VW, :], v[:, jt, h, :], ex[:, jt, :],
                            start=(jt == 0), stop=(jt == NJ - 1),
                        )
                    rc = rpool.tile([1, 512], F32R)
                    with nc.allow_low_precision(reason="fp32r reciprocal"):
                        nc.vector.reciprocal(rc[:], pc[64:65, :])
                    pr = pss.tile([64, 512], F32, tag="s")
                    nc.tensor.matmul(pr[:], ones[:], rc[:], start=True, stop=True)
                    rb = opool.tile([64, 512], F32, tag="rb")
                    nc.vector.tensor_copy(rb[:], pr[:])
                    o = opool.tile([64, 512], F32, tag="o")
                    nc.vector.tensor_tensor(
                        o[:], pc[0:64, :], rb[:], op=mybir.AluOpType.mult
                    )
                    nc.sync.dma_start(out_d[h, :, ts(ic, 512)], o[:])

    nc.compile()
    return nc


def _get_nc(use_mask: bool):
    if use_mask not in _cache:
        _cache[use_mask] = _build(use_mask)
    return _cache[use_mask]


def kernel(hidden_states, attention_mask, Wq, bq, Wk, bk, Wv, bv):
    global last_results
    hidden_states = np.asarray(hidden_states, dtype=np.float32)
    attention_mask = np.asarray(attention_mask, dtype=np.float32)
    Wq = np.asarray(Wq, dtype=np.float32)
    Wk = np.asarray(Wk, dtype=np.float32)
    Wv = np.asarray(Wv, dtype=np.float32)
    bq = np.asarray(bq, dtype=np.float32)
    bk = np.asarray(bk, dtype=np.float32)
    bv = np.asarray(bv, dtype=np.float32)

    use_mask = bool(np.any(attention_mask))
    nc = _get_nc(use_mask)

    in_maps = []
    for c in range(8):
        b = c // 2
        hg = c % 2
        cs = slice(hg * HPC * HEAD_DIM, (hg + 1) * HPC * HEAD_DIM)

        xT = np.zeros((KC * 128, S), dtype=np.float32)
        xT[:HIDDEN] = hidden_states[b].T
        xT[HIDDEN] = 1.0

        def wslice(W, bias):
            w = np.zeros((KC * 128, HPC * HEAD_DIM), dtype=np.float32)
            w[:HIDDEN] = W[:, cs]
            w[HIDDEN] = bias[cs]
            return w

        m = {
            "xT": xT,
            "cst": np.ones((1, HEAD_DIM), dtype=np.float32),
            "wq": wslice(Wq, bq),
            "wk": wslice(Wk, bk),
            "wv": wslice(Wv, bv),
        }
        if use_mask:
            em = np.exp(attention_mask[b, 0, 0, :]).astype(np.float32)
            m["em"] = np.ascontiguousarray(em.reshape(NJ, 128).T)
        in_maps.append(m)

    res = run_bass_kernel_spmd(nc, in_maps, list(range(8)))
    last_results = res

    out = np.empty((B, S, HIDDEN), dtype=np.float32)
    for c in range(8):
        b = c // 2
        hg = c % 2
        r = res.results[c]["out"]  # [6, 64, 2048]
        out[b, :, hg * HPC * HEAD_DIM:(hg + 1) * HPC * HEAD_DIM] = (
            r.transpose(2, 0, 1).reshape(S, HPC * HEAD_DIM)
        )
    return out


# revision 7
# speedup vs baseline: 1.4382x; 1.3590x over previous
"""BERT self-attention (B=4, S=2048, H=768, 12 heads x d=64) on 8 Trainium2
NeuronCores.

Sharding: core c handles batch b = c//2 and head group hg = c%2 (6 heads).
Each core computes q/k/v projections for its 6 heads from its batch's
hidden_states, then attention per head. No cross-core communication; the host
scatters inputs and gathers/reassembles the output.

Per-core layouts (SBUF is [128 partitions x free]):
  xT   [128, 7, 2048]  x[b].T padded to 896 rows; row 768 = ones (bias fold)
  wq/wk/wv [128, 7, 384]  weight column-slices, row 768 = bias
  qT/kT [128, 3, 2048]  per head-pair stacked d-dims (head even: p0-63,
                        odd: p64-127); computed as W.T @ x.T via PE
  v    [128, 16, 6, 65] token-major v per j-tile; col 64 of each head = ones
                        (yields sumexp for free during the ctx matmul)
  scoresT psum [128 j, 2, 512 i] -> exp -> expT sbuf [128, 16, 512] bf16
  ctx^T psum [65, 512]: rows 0-63 = unnormalized ctx^T, row 64 = sumexp;
  reciprocal -> PE outer-product broadcast into rows 64-127 -> DVE multiply.

Matmul dtypes: projections and scores in float32r (full-rate, ~tf32
precision); probs (expT) and v in bf16 for the ctx matmul.
"""
import numpy as np

import concourse.bass as bass
import concourse.mybir as mybir
import concourse.tile as tile
from concourse import bacc
from concourse.bass import ts
from concourse.bass_utils import run_bass_kernel_spmd

F32R = mybir.dt.float32r
F32 = mybir.dt.float32
BF16 = mybir.dt.bfloat16

HIDDEN = 768
N_HEADS = 12
HEAD_DIM = 64
B = 4
S = 2048
HPC = 6          # heads per core
KC = 7           # contraction chunks of 128 (768 data + bias row + pad)
NI = S // 512    # 4 i-chunks of 512
NJ = S // 128    # 16 j-tiles of 128
VW = 96  # v (64) | ones (1) | zeros (31): 32-aligned matmul M

_cache = {}
last_results = None


def _build(use_mask: bool):
    nc = bacc.Bacc("TRN2", target_bir_lowering=False, debug=False, num_devices=8)

    xT_d = nc.dram_tensor("xT", [KC * 128, S], F32R, kind="ExternalInput")
    wq_d = nc.dram_tensor("wq", [KC * 128, HPC * HEAD_DIM], F32R, kind="ExternalInput")
    wk_d = nc.dram_tensor("wk", [KC * 128, HPC * HEAD_DIM], F32R, kind="ExternalInput")
    wv_d = nc.dram_tensor("wv", [KC * 128, HPC * HEAD_DIM], F32R, kind="ExternalInput")
    cst_d = nc.dram_tensor("cst", [1, HEAD_DIM], F32R, kind="ExternalInput")
    if use_mask:
        em_d = nc.dram_tensor("em", [128, NJ], F32, kind="ExternalInput")
    out_d = nc.dram_tensor("out", [HPC, HEAD_DIM, S], F32, kind="ExternalOutput")

    with tile.TileContext(nc) as tc:
        with (
            tc.tile_pool(name="const", bufs=1) as cpool,
            tc.tile_pool(name="qk", bufs=1) as qkpool,
            tc.tile_pool(name="vp", bufs=1) as vpool,
            tc.tile_pool(name="op", bufs=3) as opool,
            tc.tile_pool(name="rp", bufs=2) as rpool,
            tc.tile_pool(name="pss", bufs=3, space="PSUM") as pss,
            tc.tile_pool(name="psc", bufs=2, space="PSUM") as psc,
        ):
            ones = cpool.tile([1, HEAD_DIM], F32R)
            nc.sync.dma_start(ones[:], cst_d[:])
            if use_mask:
                em = cpool.tile([128, NJ], F32)
                nc.sync.dma_start(em[:], em_d[:])

            qT = qkpool.tile([128, HPC // 2, S], BF16)
            kT = qkpool.tile([128, HPC // 2, S], BF16)
            v = vpool.tile([128, NJ, HPC, VW], BF16)
            nc.vector.memset(v[:, :, :, HEAD_DIM:HEAD_DIM + 1], 1.0)
            nc.vector.memset(v[:, :, :, HEAD_DIM + 1:VW], 0.0)

            with tc.tile_pool(name="xw", bufs=1) as xwpool:
                xT = xwpool.tile([128, KC, S], F32R)
                wq = xwpool.tile([128, KC, HPC * HEAD_DIM], F32R)
                wk = xwpool.tile([128, KC, HPC * HEAD_DIM], F32R)
                wv = xwpool.tile([128, KC, HPC * HEAD_DIM], F32R)
                for c in range(KC):
                    nc.sync.dma_start(xT[:, c, :], xT_d[ts(c, 128), :])
                for c in range(KC):
                    nc.sync.dma_start(wq[:, c, :], wq_d[ts(c, 128), :])
                    nc.sync.dma_start(wk[:, c, :], wk_d[ts(c, 128), :])
                    nc.sync.dma_start(wv[:, c, :], wv_d[ts(c, 128), :])

                # q/k projections: psum [128 (pair d-dims), 512 tokens]
                for p in range(HPC // 2):
                    for n in range(S // 512):
                        pq = psc.tile([128, 512], F32, tag="c")
                        for c in range(KC):
                            nc.tensor.matmul(
                                pq[:], wq[:, c, ts(p, 128)], xT[:, c, ts(n, 512)],
                                start=(c == 0), stop=(c == KC - 1),
                            )
                        nc.vector.tensor_copy(qT[:, p, ts(n, 512)], pq[:])
                        pk = psc.tile([128, 512], F32, tag="c")
                        for c in range(KC):
                            nc.tensor.matmul(
                                pk[:], wk[:, c, ts(p, 128)], xT[:, c, ts(n, 512)],
                                start=(c == 0), stop=(c == KC - 1),
                            )
                        nc.vector.tensor_copy(kT[:, p, ts(n, 512)], pk[:])

                # v projection: psum [128 tokens, 384]
                for jt in range(NJ):
                    pv = psc.tile([128, HPC * HEAD_DIM], F32, tag="c")
                    for c in range(KC):
                        nc.tensor.matmul(
                            pv[:], xT[:, c, ts(jt, 128)], wv[:, c, :],
                            start=(c == 0), stop=(c == KC - 1),
                        )
                    nc.vector.tensor_copy(
                        v[:, jt, :, 0:HEAD_DIM],
                        pv[:].rearrange("p (h e) -> p h e", h=HPC),
                    )

            # attention, one head-pair at a time; the two heads' K=64 scores
            # matmuls occupy disjoint PE row groups (partitions 0-63 / 64-127)
            # and run concurrently.
            with tc.tile_pool(name="ex", bufs=2) as expool:
                for pr_ in range(HPC // 2):
                    for ic in range(NI):
                        ex = expool.tile([128, NJ, 2, 512], BF16, tag="e")
                        for jt in range(NJ):
                            ss = pss.tile([128, 2, 512], F32, tag="s")
                            for a in range(2):
                                po = 64 * a
                                nc.tensor.matmul(
                                    ss[:, a, :],
                                    kT[po:po + 64, pr_, ts(jt, 128)],
                                    qT[po:po + 64, pr_, ts(ic, 512)],
                                    start=True, stop=True,
                                )
                            nc.scalar.activation(
                                ex[:, jt, :, :], ss[:],
                                mybir.ActivationFunctionType.Exp,
                                scale=1.0 / np.sqrt(HEAD_DIM),
                            )
                            if use_mask:
                                for a in range(2):
                                    nc.vector.tensor_scalar_mul(
                                        ex[:, jt, a, :], ex[:, jt, a, :],
                                        em[:, jt:jt + 1],
                                    )
                        for a in range(2):
                            h = 2 * pr_ + a
                            pc = psc.tile([128, 512], F32, tag="c")
                            for jt in range(NJ):
                                nc.tensor.matmul(
                                    pc[0:VW, :], v[:, jt, h, :], ex[:, jt, a, :],
                                    start=(jt == 0), stop=(jt == NJ - 1),
                                )
                            rc = rpool.tile([1, 512], F32R)
                            with nc.allow_low_precision(reason="fp32r reciprocal"):
                                nc.vector.reciprocal(rc[:], pc[64:65, :])
                            pr2 = pss.tile([64, 512], F32, tag="s")
                            nc.tensor.matmul(pr2[:], ones[:], rc[:], start=True, stop=True)
                            rb = opool.tile([64, 512], F32, tag="rb")
                            nc.vector.tensor_copy(rb[:], pr2[:])
                            o = opool.tile([64, 512], F32, tag="o")
                            nc.vector.tensor_tensor(
                                o[:], pc[0:64, :], rb[:], op=mybir.AluOpType.mult
                            )
                            nc.sync.dma_start(out_d[h, :, ts(ic, 512)], o[:])

    nc.compile()
    return nc


def _get_nc(use_mask: bool):
    if use_mask not in _cache:
        _cache[use_mask] = _build(use_mask)
    return _cache[use_mask]


def kernel(hidden_states, attention_mask, Wq, bq, Wk, bk, Wv, bv):
    global last_results
    hidden_states = np.asarray(hidden_states, dtype=np.float32)
    attention_mask = np.asarray(attention_mask, dtype=np.float32)
    Wq = np.asarray(Wq, dtype=np.float32)
    Wk = np.asarray(Wk, dtype=np.float32)
    Wv = np.asarray(Wv, dtype=np.float32)
    bq = np.asarray(bq, dtype=np.float32)
    bk = np.asarray(bk, dtype=np.float32)
    bv = np.asarray(bv, dtype=np.float32)

    use_mask = bool(np.any(attention_mask))
    nc = _get_nc(use_mask)

    in_maps = []
    for c in range(8):
        b = c // 2
        hg = c % 2
        cs = slice(hg * HPC * HEAD_DIM, (hg + 1) * HPC * HEAD_DIM)

        xT = np.zeros((KC * 128, S), dtype=np.float32)
        xT[:HIDDEN] = hidden_states[b].T
        xT[HIDDEN] = 1.0

        def wslice(W, bias):
            w = np.zeros((KC * 128, HPC * HEAD_DIM), dtype=np.float32)
            w[:HIDDEN] = W[:, cs]
            w[HIDDEN] = bias[cs]
            return w

        m = {
            "xT": xT,
            "cst": np.ones((1, HEAD_DIM), dtype=np.float32),
            "wq": wslice(Wq, bq),
            "wk": wslice(Wk, bk),
            "wv": wslice(Wv, bv),
        }
        if use_mask:
            em = np.exp(attention_mask[b, 0, 0, :]).astype(np.float32)
            m["em"] = np.ascontiguousarray(em.reshape(NJ, 128).T)
        in_maps.append(m)

    res = run_bass_kernel_spmd(nc, in_maps, list(range(8)))
    last_results = res

    out = np.empty((B, S, HIDDEN), dtype=np.float32)
    for c in range(8):
        b = c // 2
        hg = c % 2
        r = res.results[c]["out"]  # [6, 64, 2048]
        out[b, :, hg * HPC * HEAD_DIM:(hg + 1) * HPC * HEAD_DIM] = (
            r.transpose(2, 0, 1).reshape(S, HPC * HEAD_DIM)
        )
    return out


# revision 8
# speedup vs baseline: 1.5171x; 1.0549x over previous
"""BERT self-attention (B=4, S=2048, H=768, 12 heads x d=64) on 8 Trainium2
NeuronCores.

Sharding: core c handles batch b = c//2 and head group hg = c%2 (6 heads).
Each core computes q/k/v projections for its 6 heads from its batch's
hidden_states, then attention per head. No cross-core communication; the host
scatters inputs and gathers/reassembles the output.

Per-core layouts (SBUF is [128 partitions x free]):
  xT   [128, 7, 2048]  x[b].T padded to 896 rows; row 768 = ones (bias fold)
  wq/wk/wv [128, 7, 384]  weight column-slices, row 768 = bias
  qT/kT [128, 3, 2048]  per head-pair stacked d-dims (head even: p0-63,
                        odd: p64-127); computed as W.T @ x.T via PE
  v    [128, 16, 6, 65] token-major v per j-tile; col 64 of each head = ones
                        (yields sumexp for free during the ctx matmul)
  scoresT psum [128 j, 2, 512 i] -> exp -> expT sbuf [128, 16, 512] bf16
  ctx^T psum [65, 512]: rows 0-63 = unnormalized ctx^T, row 64 = sumexp;
  reciprocal -> PE outer-product broadcast into rows 64-127 -> DVE multiply.

Matmul dtypes: projections and scores in float32r (full-rate, ~tf32
precision); probs (expT) and v in bf16 for the ctx matmul.
"""
import numpy as np

import concourse.bass as bass
import concourse.mybir as mybir
import concourse.tile as tile
from concourse import bacc
from concourse.bass import ts
from concourse.bass_utils import run_bass_kernel_spmd

F32R = mybir.dt.float32r
F32 = mybir.dt.float32
BF16 = mybir.dt.bfloat16

HIDDEN = 768
N_HEADS = 12
HEAD_DIM = 64
B = 4
S = 2048
HPC = 6          # heads per core
KC = 7           # contraction chunks of 128 (768 data + bias row + pad)
NI = S // 512    # 4 i-chunks of 512
NJ = S // 128    # 16 j-tiles of 128
VW = 96  # v (64) | ones (1) | zeros (31): 32-aligned matmul M

_cache = {}
last_results = None


def _build(use_mask: bool):
    nc = bacc.Bacc("TRN2", target_bir_lowering=False, debug=False, num_devices=8)

    xT_d = nc.dram_tensor("xT", [KC * 128, S], F32R, kind="ExternalInput")
    wq_d = nc.dram_tensor("wq", [KC * 128, HPC * HEAD_DIM], F32R, kind="ExternalInput")
    wk_d = nc.dram_tensor("wk", [KC * 128, HPC * HEAD_DIM], F32R, kind="ExternalInput")
    wv_d = nc.dram_tensor("wv", [KC * 128, HPC * HEAD_DIM], F32R, kind="ExternalInput")
    cst_d = nc.dram_tensor("cst", [1, HEAD_DIM], F32R, kind="ExternalInput")
    if use_mask:
        em_d = nc.dram_tensor("em", [128, NJ], F32, kind="ExternalInput")
    out_d = nc.dram_tensor("out", [HPC, HEAD_DIM, S], F32, kind="ExternalOutput")

    with tile.TileContext(nc) as tc:
        with (
            tc.tile_pool(name="const", bufs=1) as cpool,
            tc.tile_pool(name="qk", bufs=1) as qkpool,
            tc.tile_pool(name="vp", bufs=1) as vpool,
            tc.tile_pool(name="op", bufs=3) as opool,
            tc.tile_pool(name="rp", bufs=2) as rpool,
        ):
            ones = cpool.tile([1, HEAD_DIM], F32R)
            nc.sync.dma_start(ones[:], cst_d[:])
            if use_mask:
                em = cpool.tile([128, NJ], F32)
                nc.sync.dma_start(em[:], em_d[:])

            qT = qkpool.tile([128, HPC // 2, S], BF16)
            kT = qkpool.tile([128, HPC // 2, S], BF16)
            v = vpool.tile([128, NJ, HPC, VW], BF16)
            nc.vector.memset(v[:, :, :, HEAD_DIM:HEAD_DIM + 1], 1.0)
            nc.vector.memset(v[:, :, :, HEAD_DIM + 1:VW], 0.0)

            with (
                tc.tile_pool(name="xw", bufs=1) as xwpool,
                tc.tile_pool(name="qkv4", bufs=2, space="PSUM") as qkv4,
            ):
                xT = xwpool.tile([128, KC, S], F32R)
                wq = xwpool.tile([128, KC, HPC * HEAD_DIM], F32R)
                wk = xwpool.tile([128, KC, HPC * HEAD_DIM], F32R)
                wv = xwpool.tile([128, KC, HPC * HEAD_DIM], F32R)
                for c in range(KC):
                    nc.sync.dma_start(xT[:, c, :], xT_d[ts(c, 128), :])
                for c in range(KC):
                    nc.sync.dma_start(wv[:, c, :], wv_d[ts(c, 128), :])
                    nc.sync.dma_start(wq[:, c, :], wq_d[ts(c, 128), :])
                    nc.sync.dma_start(wk[:, c, :], wk_d[ts(c, 128), :])

                # v projection first (unblocks attention on pair 0 sooner):
                # psum [128 tokens, 384]
                for jt in range(NJ):
                    pv = qkv4.tile([128, HPC * HEAD_DIM], F32, tag="a")
                    for c in range(KC):
                        nc.tensor.matmul(
                            pv[:], xT[:, c, ts(jt, 128)], wv[:, c, :],
                            start=(c == 0), stop=(c == KC - 1),
                        )
                    nc.vector.tensor_copy(
                        v[:, jt, :, 0:HEAD_DIM],
                        pv[:].rearrange("p (h e) -> p h e", h=HPC),
                    )

                # q/k projections: c-outer so the stationary weight chunk is
                # loaded once per 4 token-chunk matmuls; accumulate in a
                # 4-bank psum group [128 (pair d-dims), 4, 512].
                for p in range(HPC // 2):
                    for w_, dst in ((wq, qT), (wk, kT)):
                        acc = qkv4.tile([128, S // 512, 512], F32, tag="a")
                        for c in range(KC):
                            for n in range(S // 512):
                                nc.tensor.matmul(
                                    acc[:, n, :], w_[:, c, ts(p, 128)],
                                    xT[:, c, ts(n, 512)],
                                    start=(c == 0), stop=(c == KC - 1),
                                )
                        nc.vector.tensor_copy(
                            dst[:, p, :], acc[:].rearrange("p a n -> p (a n)")
                        )

            # attention, one head-pair at a time; the two heads' K=64 scores
            # matmuls occupy disjoint PE row groups (partitions 0-63 / 64-127)
            # and run concurrently.
            with (
                tc.tile_pool(name="ex", bufs=2) as expool,
                tc.tile_pool(name="pss", bufs=3, space="PSUM") as pss,
                tc.tile_pool(name="psc", bufs=2, space="PSUM") as psc,
            ):
                for pr_ in range(HPC // 2):
                    for ic in range(NI):
                        ex = expool.tile([128, NJ, 2, 512], BF16, tag="e")
                        for jt in range(NJ):
                            ss = pss.tile([128, 2, 512], F32, tag="s")
                            for a in range(2):
                                po = 64 * a
                                nc.tensor.matmul(
                                    ss[:, a, :],
                                    kT[po:po + 64, pr_, ts(jt, 128)],
                                    qT[po:po + 64, pr_, ts(ic, 512)],
                                    start=True, stop=True,
                                )
                            nc.scalar.activation(
                                ex[:, jt, :, :], ss[:],
                                mybir.ActivationFunctionType.Exp,
                                scale=1.0 / np.sqrt(HEAD_DIM),
                            )
                            if use_mask:
                                for a in range(2):
                                    nc.vector.tensor_scalar_mul(
                                        ex[:, jt, a, :], ex[:, jt, a, :],
                                        em[:, jt:jt + 1],
                                    )
                        for a in range(2):
                            h = 2 * pr_ + a
                            pc = psc.tile([128, 512], F32, tag="c")
                            for jt in range(NJ):
                                nc.tensor.matmul(
                                    pc[0:VW, :], v[:, jt, h, :], ex[:, jt, a, :],
                                    start=(jt == 0), stop=(jt == NJ - 1),
                                )
                            rc = rpool.tile([1, 512], F32)
                            nc.vector.reciprocal(rc[:], pc[64:65, :])
                            rb = opool.tile([64, 512], F32, tag="rb")
                            nc.gpsimd.partition_broadcast(rb[:], rc[:])
                            o = opool.tile([64, 512], F32, tag="o")
                            nc.vector.tensor_tensor(
                                o[:], pc[0:64, :], rb[:], op=mybir.AluOpType.mult
                            )
                            nc.sync.dma_start(out_d[h, :, ts(ic, 512)], o[:])

    nc.compile()
    return nc


def _get_nc(use_mask: bool):
    if use_mask not in _cache:
        _cache[use_mask] = _build(use_mask)
    return _cache[use_mask]


def kernel(hidden_states, attention_mask, Wq, bq, Wk, bk, Wv, bv):
    global last_results
    hidden_states = np.asarray(hidden_states, dtype=np.float32)
    attention_mask = np.asarray(attention_mask, dtype=np.float32)
    Wq = np.asarray(Wq, dtype=np.float32)
    Wk = np.asarray(Wk, dtype=np.float32)
    Wv = np.asarray(Wv, dtype=np.float32)
    bq = np.asarray(bq, dtype=np.float32)
    bk = np.asarray(bk, dtype=np.float32)
    bv = np.asarray(bv, dtype=np.float32)

    use_mask = bool(np.any(attention_mask))
    nc = _get_nc(use_mask)

    in_maps = []
    for c in range(8):
        b = c // 2
        hg = c % 2
        cs = slice(hg * HPC * HEAD_DIM, (hg + 1) * HPC * HEAD_DIM)

        xT = np.zeros((KC * 128, S), dtype=np.float32)
        xT[:HIDDEN] = hidden_states[b].T
        xT[HIDDEN] = 1.0

        def wslice(W, bias):
            w = np.zeros((KC * 128, HPC * HEAD_DIM), dtype=np.float32)
            w[:HIDDEN] = W[:, cs]
            w[HIDDEN] = bias[cs]
            return w

        m = {
            "xT": xT,
            "cst": np.ones((1, HEAD_DIM), dtype=np.float32),
            "wq": wslice(Wq, bq),
            "wk": wslice(Wk, bk),
            "wv": wslice(Wv, bv),
        }
        if use_mask:
            em = np.exp(attention_mask[b, 0, 0, :]).astype(np.float32)
            m["em"] = np.ascontiguousarray(em.reshape(NJ, 128).T)
        in_maps.append(m)

    res = run_bass_kernel_spmd(nc, in_maps, list(range(8)))
    last_results = res

    out = np.empty((B, S, HIDDEN), dtype=np.float32)
    for c in range(8):
        b = c // 2
        hg = c % 2
        r = res.results[c]["out"]  # [6, 64, 2048]
        out[b, :, hg * HPC * HEAD_DIM:(hg + 1) * HPC * HEAD_DIM] = (
            r.transpose(2, 0, 1).reshape(S, HPC * HEAD_DIM)
        )
    return out
